# revision 15
# baseline (speedup 1.0000x reference)
"""BLiqNet (liquid-ODE MLP, single RK4 step) Trainium2 kernel.

Math (reference, fp32):
    u  = x @ Wx.T + bx                  # [B, H=128]
    uu = u @ Wu.T + b_ode
    f(h) = -h + tanh(h @ Wh.T + uu)
    RK4 with dt=2.0 from h0=u:  g_i = tanh(h_i @ Wh.T + uu)
      h2 = g1 ; h3 = h0 - g1 + g2 ; h4 = -h0 + 2 g1 - 2 g2 + 2 g3
      h_out = (h0 - g1 + 2 g2 + g4) / 3
    y = h_out @ Wout.T + bout           # [B, 256]

Strategy: pure data parallel over 8 cores (batch sharded).  Everything on
chip runs in "T layout" (features on partitions, batch on the free dim):
x is transposed per shard on the host and y is transposed back after.
All matmuls run as float32r (full-rate single-pass fp32 PE path).  The RK4
combination is algebraically expanded so every stage is a plain matmul
accumulation into PSUM with host-precomputed combined weights; biases fold
into the per-partition ACT bias operand of the tanh that reads each PSUM;
the h_out combination runs on DVE with fused scalar_tensor_tensor ops.

The per-tile dependency chain (4 chained matmul+tanh stages) is ~14 us,
far above the per-tile DMA budget, so the emission loop is an explicit
software pipeline: each outer step emits a different pipeline stage for a
different batch tile, giving every engine independent work every step.
Per step (one 512-column tile retired): 13 matmuls on PE, 4 tanh + 2
biased-identity PSUM reads on ACT, 5 elementwise ops on DVE, 1.5 MB of
DMA.  PSUM is planned to exactly 8 banks.
"""

import sys

sys.path.insert(0, "/opt/trn_rl_repo")

import numpy as np

from contextlib import ExitStack

import concourse.bacc as bacc
import concourse.tile as tile
from concourse import bass_utils, mybir

F32 = mybir.dt.float32
F32R = mybir.dt.float32r
AF = mybir.ActivationFunctionType
ALU = mybir.AluOpType

B, D_IN, H, D_OUT = 262144, 512, 128, 256
N_CORES = 8
B_CORE = B // N_CORES

# weight-stack slot indices (stack is [128, NW, 128] in DRAM)
NW = 14
(WX0, WX1, WX2, WX3, SL1, SLWU, SLWH, SLWHN, SL4U, SLWH2,
 SWO0, SWO1, SI, SIN) = range(NW)
NB = 5  # bias-stack columns: b1, b2, b4, by0, by1
_NC_CACHE: dict = {}


def _prep_weights(Wx, bx, Wh, Wu, b_ode, Wout, bout):
    """Pack all matmul lhsT blocks into one [128, NW, 128] stack plus a
    [128, NB] bias stack.  lhsT convention: out = lhsT.T @ rhs, so for
    pre = M @ g the block is M.T."""
    f = np.float32
    Wx, bx, Wh, Wu = Wx.astype(f), bx.astype(f), Wh.astype(f), Wu.astype(f)
    b_ode, Wout, bout = b_ode.astype(f), Wout.astype(f), bout.astype(f)

    ws = np.zeros((128, NW, 128), dtype=f)
    WxT = Wx.T  # [512, H]
    for k in range(4):
        ws[:, WX0 + k, :] = WxT[k * 128:(k + 1) * 128, :]
    ws[:, SL1, :] = (Wh + Wu).T
    ws[:, SLWU, :] = Wu.T
    ws[:, SLWH, :] = Wh.T
    ws[:, SLWHN, :] = (-Wh).T
    ws[:, SL4U, :] = (Wu - Wh).T
    ws[:, SLWH2, :] = (2.0 * Wh).T
    WoT3 = (Wout / 3.0).T  # [128, 256]
    ws[:, SWO0, :] = WoT3[:, 0:128]
    ws[:, SWO1, :] = WoT3[:, 128:256]
    eye = np.eye(128, dtype=f)
    ws[:, SI, :] = eye
    ws[:, SIN, :] = -eye

    bs = np.zeros((128, NB), dtype=f)
    bs[:, 0] = (Wh + Wu) @ bx + b_ode        # b1 (also b3)
    bs[:, 1] = Wu @ bx + b_ode               # b2
    bs[:, 2] = (Wu - Wh) @ bx + b_ode        # b4
    by = (Wout @ bx) / 3.0 + bout            # [256]
    bs[:, 3] = by[0:128]
    bs[:, 4] = by[128:256]
    return ws, bs


def _build(b_core: int, n_tile: int = 512):
    """Build + compile the per-core Tile kernel (SPMD across cores)."""
    assert n_tile == 512
    nc = bacc.Bacc("TRN2", target_bir_lowering=False, debug=False)

    xT_d = nc.dram_tensor("xT", [D_IN, b_core], F32R, kind="ExternalInput")
    ws_d = nc.dram_tensor("ws", [128, NW, 128], F32R, kind="ExternalInput")
    bs_d = nc.dram_tensor("bs", [128, NB], F32, kind="ExternalInput")
    yT_d = nc.dram_tensor("yT", [D_OUT, b_core], F32, kind="ExternalOutput")

    xT_r = xT_d.rearrange("(k p) n -> p k n", p=128)  # [128, 4, b_core]
    yT_r = yT_d.rearrange("(h p) n -> p h n", p=128)  # [128, 2, b_core]

    n_tiles = b_core // n_tile

    with tile.TileContext(nc) as tc, ExitStack() as ctx:
        cpool = ctx.enter_context(tc.tile_pool(name="const", bufs=1))
        xpool = ctx.enter_context(tc.tile_pool(name="x", bufs=2))
        x1pool = ctx.enter_context(tc.tile_pool(name="x1", bufs=4))
        upool = ctx.enter_context(tc.tile_pool(name="u", bufs=7))
        g1pool = ctx.enter_context(tc.tile_pool(name="g1", bufs=6))
        g2pool = ctx.enter_context(tc.tile_pool(name="g2", bufs=5))
        g3pool = ctx.enter_context(tc.tile_pool(name="g3", bufs=2))
        g4pool = ctx.enter_context(tc.tile_pool(name="g4", bufs=3))
        tpool = ctx.enter_context(tc.tile_pool(name="t", bufs=3))
        gspool = ctx.enter_context(tc.tile_pool(name="gs", bufs=3))
        s2pool = ctx.enter_context(tc.tile_pool(name="s24", bufs=3))
        hpool = ctx.enter_context(tc.tile_pool(name="hs", bufs=3))
        ypool = ctx.enter_context(tc.tile_pool(name="y", bufs=2))
        # PSUM: 8 banks exactly
        pu_pool = ctx.enter_context(tc.tile_pool(name="pu", bufs=1, space="PSUM"))
        pa_pool = ctx.enter_context(tc.tile_pool(name="pa", bufs=3, space="PSUM"))
        pb_pool = ctx.enter_context(tc.tile_pool(name="pb", bufs=1, space="PSUM"))
        p4_pool = ctx.enter_context(tc.tile_pool(name="p4", bufs=1, space="PSUM"))
        ph_pool = ctx.enter_context(tc.tile_pool(name="ph", bufs=1, space="PSUM"))
        py_pool = ctx.enter_context(tc.tile_pool(name="py", bufs=1, space="PSUM"))

        ws_sb = cpool.tile([128, NW, 128], F32R)
        nc.sync.dma_start(ws_sb[:], ws_d[:])
        bs_sb = cpool.tile([128, NB], F32)
        nc.sync.dma_start(bs_sb[:], bs_d[:])

        def W(j):
            return ws_sb[:, j, :]

        def bias(j):
            return bs_sb[:, j:j + 1]

        # live per-tile SBUF tiles, keyed by tile index
        xs, us, g1s, g2s, g3s, g4s, ts, gss, t1s, hss, ys = (
            {} for _ in range(11))
        GRP = 4   # tiles per x-load DMA (4 MB)
        YGRP = 4  # tiles per y-store DMA (2 MB)

        def s0_load(i):  # x prefetch
            if i < GRP:
                # first group: per-tile 1 MB DMAs so the pipeline starts fast
                xc = x1pool.tile([128, 4, 512], F32R, tag="x1", name="x1c")
                nc.sync.dma_start(xc[:], xT_r[:, :, i * 512:(i + 1) * 512])
                xs[i] = (xc, 0)
                return
            if i % GRP:
                return
            xc = xpool.tile([128, 4, GRP * 512], F32R, tag="x", name="xc")
            nc.sync.dma_start(xc[:], xT_r[:, :, i * 512:(i + GRP) * 512])
            for j in range(GRP):
                xs[i + j] = (xc, j * 512)

        def s1_u(i):  # u matmuls + copy to SBUF
            pu = pu_pool.tile([128, 512], F32, tag="pu", name="pu")
            xc, c0 = xs.pop(i)
            for k in range(4):
                nc.tensor.matmul(pu[:], W(WX0 + k), xc[:, k, c0:c0 + 512],
                                 start=(k == 0), stop=(k == 3))
            us[i] = upool.tile([128, 512], F32R, tag="u", name="ut")
            nc.vector.tensor_copy(us[i][:], pu[:])

        def s2_g1(i):  # pre1 = (Wh+Wu) @ u ; g1 = tanh(+b1)
            pa = pa_pool.tile([128, 512], F32, tag="pa", name="pa")
            nc.tensor.matmul(pa[:], W(SL1), us[i][:], start=True, stop=False)
            g1s[i] = (g1pool.tile([128, 512], F32R, tag="g1", name="g1t"), pa)
            nc.scalar.activation(g1s[i][0][:], pa[:], AF.Tanh, bias=bias(0))

        def s3_g2(i):  # pre2 = Wu@u + Wh@g1 ; g2 ; t = g2 - g1 ; t1 = u - g1
            pb = pb_pool.tile([128, 512], F32, tag="pb", name="pb")
            nc.tensor.matmul(pb[:], W(SLWU), us[i][:], start=True, stop=False)
            nc.tensor.matmul(pb[:], W(SLWH), g1s[i][0][:], start=False, stop=True)
            g2s[i] = g2pool.tile([128, 512], F32R, tag="g2", name="g2t")
            nc.scalar.activation(g2s[i][:], pb[:], AF.Tanh, bias=bias(1))
            ts[i] = tpool.tile([128, 512], F32R, tag="t", name="tt")
            nc.vector.scalar_tensor_tensor(ts[i][:], g1s[i][0][:], -1.0,
                                           g2s[i][:], ALU.mult, ALU.add)

        def s4_g3(i):  # pre3 += -Wh@g1 + Wh@g2 (onto pre1 psum) ; g3 ; gsum
            g1, pa = g1s[i]
            nc.tensor.matmul(pa[:], W(SLWHN), g1[:], start=False, stop=False)
            nc.tensor.matmul(pa[:], W(SLWH), g2s[i][:], start=False, stop=True)
            g3 = g3pool.tile([128, 512], F32R, tag="g3", name="g3t")
            nc.scalar.activation(g3[:], pa[:], AF.Tanh, bias=bias(0))
            gss[i] = gspool.tile([128, 512], F32R, tag="gs", name="gst")
            nc.vector.scalar_tensor_tensor(gss[i][:], ts.pop(i)[:], -1.0,
                                           g3[:], ALU.mult, ALU.add)

        def s5_g4(i):  # pre4 = (Wu-Wh)@u + 2Wh@gsum ; g4
            p4 = p4_pool.tile([128, 512], F32, tag="p4", name="p4")
            nc.tensor.matmul(p4[:], W(SL4U), us[i][:], start=True, stop=False)
            nc.tensor.matmul(p4[:], W(SLWH2), gss.pop(i)[:],
                             start=False, stop=True)
            g4s[i] = g4pool.tile([128, 512], F32R, tag="g4", name="g4t")
            nc.scalar.activation(g4s[i][:], p4[:], AF.Tanh, bias=bias(2))

        def s6_hsum(i):  # hsum = (u - g1) [PE] + (2 g2 + g4) [DVE]
            ph = ph_pool.tile([128, 512], F32, tag="ph", name="ph")
            nc.tensor.matmul(ph[:], W(SI), us.pop(i)[:], start=True, stop=False)
            nc.tensor.matmul(ph[:], W(SIN), g1s.pop(i)[0][:],
                             start=False, stop=True)
            s24 = s2pool.tile([128, 512], F32R, tag="s24", name="s24t")
            nc.vector.scalar_tensor_tensor(s24[:], g2s.pop(i)[:], 2.0,
                                           g4s.pop(i)[:], ALU.mult, ALU.add)
            hss[i] = hpool.tile([128, 512], F32R, tag="hs", name="hst")
            nc.vector.tensor_add(hss[i][:], ph[:], s24[:])

        def s7_y(i):  # y halves + batched store (one 1 MB DMA per YGRP tiles)
            if i % YGRP == 0:
                ys[i // YGRP] = ypool.tile([128, 2, YGRP * 512], F32,
                                           tag="y", name="yt")
            y_sb = ys[i // YGRP]
            c0 = (i % YGRP) * 512
            for h in range(2):
                py = py_pool.tile([128, 512], F32, tag="py", name="py")
                nc.tensor.matmul(py[:], W(SWO0 + h), hss[i][:],
                                 start=True, stop=True)
                nc.scalar.activation(y_sb[:, h, c0:c0 + 512], py[:],
                                     AF.Identity, bias=bias(3 + h))
            del hss[i]
            if i % YGRP == YGRP - 1:
                g0 = (i // YGRP) * YGRP
                nc.sync.dma_start(yT_r[:, :, g0 * 512:(g0 + YGRP) * 512], y_sb[:])
                del ys[i // YGRP]

        stages = [s0_load, s1_u, s2_g1, s3_g2, s4_g3, s5_g4, s6_hsum, s7_y]
        offs = [0, 2, 3, 4, 5, 6, 7, 8]
        for step in range(n_tiles + offs[-1]):
            for stage, off in zip(stages, offs):
                i = step - off
                if 0 <= i < n_tiles:
                    stage(i)

    nc.compile()
    return nc


def _get_nc(b_core: int, n_tile: int):
    key = (b_core, n_tile)
    if key not in _NC_CACHE:
        _NC_CACHE[key] = _build(b_core, n_tile)
    return _NC_CACHE[key]


def _kernel_impl(x, Wx, bx, Wh, Wu, b_ode, Wout, bout,
                 n_cores=N_CORES, n_tile=512, **run_kwargs):
    b = x.shape[0]
    b_core = b // n_cores
    ws, bs = _prep_weights(Wx, bx, Wh, Wu, b_ode, Wout, bout)

    # host-side shard + transpose: [n_cores][D_IN, b_core], C-contiguous
    x = np.ascontiguousarray(x, dtype=np.float32)
    shards = x.reshape(n_cores, b_core, D_IN).transpose(0, 2, 1)

    nc = _get_nc(b_core, n_tile)
    in_maps = [
        {"xT": np.ascontiguousarray(shards[c]), "ws": ws, "bs": bs}
        for c in range(n_cores)
    ]
    res = bass_utils.run_bass_kernel_spmd(
        nc, in_maps, core_ids=list(range(n_cores)), **run_kwargs
    )
    y = np.empty((b, D_OUT), dtype=np.float32)
    for c in range(n_cores):
        y[c * b_core:(c + 1) * b_core] = res.results[c]["yT"].T
    return y, res


def kernel(x, Wx, bx, Wh, Wu, b_ode, Wout, bout):
    y, _ = _kernel_impl(x, Wx, bx, Wh, Wu, b_ode, Wout, bout)
    return y
